# revision 7
# baseline (speedup 1.0000x reference)
"""Trainium2 Bass kernel for nn_ControlFlexHNN (dense_mlp) — v2.

Data-parallel across 8 NeuronCores: batch N=32768 -> 4096 rows/core.
Feature-major on-chip layout ([feature, batch]); all matmul inputs bf16
(PSUM accumulation f32), which halves SBUF/DMA and doubles DVE rates.

Host-side folds (all into weight prep, free at runtime):
  - u = tanh(z @ Wp.T + bp); zua = [z; 1; u]  (21 rows: the ones-row
    folds b1/bf1 into the A/C matmuls)
  - w2nh = diag(Wh) @ W2  (folds the ga2 = s1*Wh scale into the D matmul)
  - w1nj/wffj/bffj = J-mapped & negated head weights (folds the final
    symplectic map J into the heads; device output is the final answer
    up to transpose)

Device kernel per core (B=512 batch tile, 8 tiles), software-pipelined
emission so the PE never waits on the Act/DVE/Pool chains:
  iter t: [E(t-1) chains + AC(t) pairs interleaved] [D(t-1) + sout(t-1)]
          [B(t)]
  A: pa[j] = w1a.T @ zua[0:17]          h1 = tanh(pa)        s0 = 1-h1^2
  C: pf[j] = wf1a.T @ zua               f1sb = copy(pf)      th = tanh(f1sb)
  B: pb[j] = sum_k w2t[k][j] @ h1[k]    h2 = tanh(pb+b2)     s1 = 1-h2^2
                                        prod = f1sb*s1       g1 = th+prod
  D: pg[k] = sum_j w2nh[j][k] @ s1[j]   ga1 = pg*s0[k]       ps += w1nj[k] @ ga1
  E: pf2[j] = sum_k wf2t[k][j] @ g1[k]  th2 = tanh(pf2+bf2)  t2 = pf2+bf2
                                        prod2 = t2*s0        g2 = th2+prod2
                                        ps += wffj[j] @ g2
  sout = ps + bffj  (f32) -> DMA out
"""

import contextlib

import numpy as np

N = 32768
DQ = 8
D2 = 2 * DQ          # 16
A_DIM = 4
ZUA = D2 + 1 + A_DIM  # 21 rows: z(16), ones(1), u(4)
ZB = D2 + 1           # 17 rows for the a1 matmul (z + bias row)
H = 1024
HC = H // 128        # 8 chunks
NCORES = 8
NSH = N // NCORES    # 4096 rows per core
B = 512              # batch tile (free dim of matmuls)
TILES = NSH // B     # 8
B2 = B // 2          # 256 half-tile for same-weight MM pairs
E_LAG = 3            # E-head j runs after chain j+E_LAG
D_LAG = 1

_BUILT = None


def _build(loop_n=None):
    import concourse.bacc as bacc
    import concourse.mybir as mybir
    from concourse import tile

    f32 = mybir.dt.float32
    bf16 = mybir.dt.bfloat16

    nc = bacc.Bacc(None)

    zua_d = nc.dram_tensor("zua", [ZUA, NSH], bf16, kind="ExternalInput")
    w1a_d = nc.dram_tensor("w1a", [ZB, H], bf16, kind="ExternalInput")
    wf1a_d = nc.dram_tensor("wf1a", [ZUA, H], bf16, kind="ExternalInput")
    w2t_d = nc.dram_tensor("w2t", [H, H], bf16, kind="ExternalInput")
    w2nh_d = nc.dram_tensor("w2nh", [H, H], bf16, kind="ExternalInput")
    wf2t_d = nc.dram_tensor("wf2t", [H, H], bf16, kind="ExternalInput")
    w1nj_d = nc.dram_tensor("w1nj", [H, D2], bf16, kind="ExternalInput")
    wffj_d = nc.dram_tensor("wffj", [H, D2], bf16, kind="ExternalInput")
    b2c_d = nc.dram_tensor("b2c", [128, HC], f32, kind="ExternalInput")
    bf2c_d = nc.dram_tensor("bf2c", [128, HC], f32, kind="ExternalInput")
    bffj_d = nc.dram_tensor("bffj", [D2, 1], f32, kind="ExternalInput")
    st_d = nc.dram_tensor("st", [D2, NSH], f32, kind="ExternalOutput")

    with tile.TileContext(nc) as tc:
        with (
            tc.tile_pool(name="wp", bufs=1) as wp,
            tc.tile_pool(name="act2", bufs=2) as act2,
            tc.tile_pool(name="tmp3", bufs=3) as tmp3,
            tc.tile_pool(name="g2p", bufs=6) as g2p,
            tc.tile_pool(name="ga1p", bufs=4) as ga1p,
            tc.tile_pool(name="iop", bufs=2) as iop,
            tc.tile_pool(name="smallp", bufs=3, space="PSUM") as smallp,
            tc.tile_pool(name="mmp", bufs=4, space="PSUM") as mmp,
            tc.tile_pool(name="accp", bufs=1, space="PSUM") as accp,
        ):
            # ---- resident weights ----
            w1a = wp.tile([ZB, H], bf16)
            nc.sync.dma_start(w1a[:], w1a_d[:])
            wf1a = wp.tile([ZUA, H], bf16)
            nc.sync.dma_start(wf1a[:], wf1a_d[:])
            w2t = wp.tile([128, HC, H], bf16)
            nc.sync.dma_start(w2t[:], w2t_d.rearrange("(c p) j -> p c j", p=128))
            w2nh = wp.tile([128, HC, H], bf16)
            nc.sync.dma_start(w2nh[:], w2nh_d.rearrange("(c p) k -> p c k", p=128))
            wf2t = wp.tile([128, HC, H], bf16)
            nc.sync.dma_start(wf2t[:], wf2t_d.rearrange("(c p) j -> p c j", p=128))
            w1nj = wp.tile([128, HC, D2], bf16)
            nc.sync.dma_start(w1nj[:], w1nj_d.rearrange("(c p) m -> p c m", p=128))
            wffj = wp.tile([128, HC, D2], bf16)
            nc.sync.dma_start(wffj[:], wffj_d.rearrange("(c p) m -> p c m", p=128))
            b2c = wp.tile([128, HC], f32)
            nc.sync.dma_start(b2c[:], b2c_d[:])
            bf2c = wp.tile([128, HC], f32)
            nc.sync.dma_start(bf2c[:], bf2c_d[:])
            bffj = wp.tile([D2, 1], f32)
            nc.sync.dma_start(bffj[:], bffj_d[:])

            weights = (w1a, wf1a, w2t, w2nh, wf2t, w1nj, wffj, b2c, bf2c, bffj)
            pools = (act2, tmp3, g2p, ga1p, iop, smallp, mmp, accp)

            loop_cm = tc.For_i(0, loop_n, 1) if loop_n else contextlib.nullcontext()
            with loop_cm:
                _emit_body(nc, mybir, pools, weights, zua_d, st_d)

    nc.compile()
    return nc


def _build_looped(loop_n):
    return _build(loop_n=loop_n)


def _emit_body(nc, mybir, pools, weights, zua_d, st_d):
    act2, tmp3, g2p, ga1p, iop, smallp, mmp, accp = pools
    w1a, wf1a, w2t, w2nh, wf2t, w1nj, wffj, b2c, bf2c, bffj = weights

    f32 = mybir.dt.float32
    bf16 = mybir.dt.bfloat16
    Tanh = mybir.ActivationFunctionType.Tanh
    Ident = mybir.ActivationFunctionType.Identity
    mult = mybir.AluOpType.mult
    add = mybir.AluOpType.add

    # per-tile state carried between phases
    zu = [None] * TILES
    h1 = [None] * TILES
    s0 = [None] * TILES
    s1 = [None] * TILES
    g1 = [None] * TILES
    ps = [None] * TILES

    def dma_in(t):
        zu[t] = iop.tile([ZUA, B], bf16, tag="zua", name=f"zua_{t}")
        nc.sync.dma_start(zu[t][:], zua_d[:, t * B:(t + 1) * B])

    def phase_A_single(t, j):
        """One pa + its h1 (interleaved into E phase)."""
        if j == 0:
            h1[t] = act2.tile([128, HC, B], bf16, tag="h1", name=f"h1_{t}")
        pa = smallp.tile([128, 2, B2], f32, tag="sm", name=f"pa_{t}_{j}")
        for i in range(2):
            nc.tensor.matmul(pa[:, i, :], w1a[:, j * 128:(j + 1) * 128],
                             zu[t][0:ZB, i * B2:(i + 1) * B2],
                             start=(i == 0), stop=(i == 1),
                             skip_group_check=True)
        nc.scalar.activation(h1[t][:, j, :], pa[:], Tanh)

    def phase_E(tp, t_next):
        """E chain-pairs for tile tp; AC pairs for t_next interleaved."""
        g2s = []

        def e_head(j):
            for i in range(2):
                nc.tensor.matmul(ps[tp][:, i * B2:(i + 1) * B2],
                                 wffj[:, j, :],
                                 g2s[j][:, i * B2:(i + 1) * B2],
                                 start=(j == 0 and i == 0), stop=False,
                                 skip_group_check=True)

        def e_post(j):
            th2 = tmp3.tile([128, B], bf16, tag="th2", name=f"th2_{tp}_{j}")
            nc.scalar.activation(th2[:], pf2s[j % 2][:], Tanh,
                                 bias=bf2c[:, j:j + 1])
            t2 = tmp3.tile([128, B], bf16, tag="t2", name=f"t2_{tp}_{j}")
            nc.vector.tensor_scalar_add(t2[:], pf2s[j % 2][:], bf2c[:, j:j + 1])
            prod2 = tmp3.tile([128, B], bf16, tag="prod2", name=f"prod2_{tp}_{j}")
            nc.vector.tensor_tensor(out=prod2[:], in0=t2[:], in1=s0[tp][:, j, :],
                                    op=mult)
            g2 = g2p.tile([128, B], bf16, tag="g2", name=f"g2_{tp}_{j}")
            nc.gpsimd.tensor_tensor(out=g2[:], in0=th2[:], in1=prod2[:], op=add)
            g2s.append(g2)

        ps[tp] = accp.tile([D2, B], f32, tag="acc", name=f"ps_{tp}")
        for jp in range(0, HC, 2):
            pf2s = []
            for jj in range(jp, jp + 2):
                pf2 = mmp.tile([128, 2, B2], f32, tag="mm",
                               name=f"pf2_{tp}_{jj}")
                for k in range(HC):
                    for i in range(2):
                        nc.tensor.matmul(
                            pf2[:, i, :], wf2t[:, k, jj * 128:(jj + 1) * 128],
                            g1[tp][:, k, i * B2:(i + 1) * B2],
                            start=(k == 0 and i == 0),
                            stop=(k == 7 and i == 1),
                            skip_group_check=True)
                pf2s.append(pf2)
            # bank-freeing consumers first, then slack-rich AC work
            e_post(jp)
            e_post(jp + 1)
            if t_next is not None:
                phase_A_single(t_next, jp)
                phase_A_single(t_next, jp + 1)
            # heads for the pair two pairs back (chunks jp-4, jp-3)
            if jp >= 4:
                e_head(jp - 4)
                e_head(jp - 3)
        for j in range(HC - 4, HC):
            e_head(j)

    def phase_D(tp):
        ga1s = []

        def d_head(k, stop):
            for i in range(2):
                nc.tensor.matmul(ps[tp][:, i * B2:(i + 1) * B2],
                                 w1nj[:, k, :],
                                 ga1s[k][:, i * B2:(i + 1) * B2],
                                 start=False, stop=(stop and i == 1),
                                 skip_group_check=True)

        def d_post(k, pg):
            ga1 = ga1p.tile([128, B], bf16, tag="ga1", name=f"ga1_{tp}_{k}")
            nc.vector.tensor_tensor(out=ga1[:], in0=pg[:], in1=s0[tp][:, k, :],
                                    op=mult)
            ga1s.append(ga1)
            # spread next tile's s0 computation into this phase (late deadline)
            tn = tp + 1
            if tn < TILES and h1[tn] is not None:
                nc.vector.tensor_tensor(out=s0[tn][:, k, :], in0=h1[tn][:, k, :],
                                        in1=h1[tn][:, k, :], op=mult)
                nc.vector.tensor_scalar(out=s0[tn][:, k, :], in0=s0[tn][:, k, :],
                                        scalar1=-1.0, scalar2=1.0,
                                        op0=mult, op1=add)

        for kp in range(0, HC, 2):
            pgs = []
            for kk in range(kp, kp + 2):
                pg = mmp.tile([128, 2, B2], f32, tag="mm", name=f"pg_{tp}_{kk}")
                for j in range(HC):
                    for i in range(2):
                        nc.tensor.matmul(
                            pg[:, i, :], w2nh[:, j, kk * 128:(kk + 1) * 128],
                            s1[tp][:, j, i * B2:(i + 1) * B2],
                            start=(j == 0 and i == 0),
                            stop=(j == 7 and i == 1),
                            skip_group_check=True)
                pgs.append(pg)
            d_post(kp, pgs[0])
            d_post(kp + 1, pgs[1])
            if kp >= 2:
                d_head(kp - 2, stop=False)
                d_head(kp - 1, stop=False)
        for k in range(HC - 2, HC):
            d_head(k, stop=(k == HC - 1))

        sout = iop.tile([D2, B], f32, tag="sout", name=f"sout_{tp}")
        nc.scalar.activation(sout[:], ps[tp][:], Ident, bias=bffj[:, 0:1])
        nc.sync.dma_start(st_d[:, tp * B:(tp + 1) * B], sout[:])

    def phase_B(t):
        s1[t] = act2.tile([128, HC, B], bf16, tag="s1", name=f"s1_{t}")
        g1[t] = act2.tile([128, HC, B], bf16, tag="g1", name=f"g1_{t}")

        def b_post(j, pb, pf):
            h2t = tmp3.tile([128, B], bf16, tag="h2t", name=f"h2t_{t}_{j}")
            nc.scalar.activation(h2t[:], pb[:], Tanh, bias=b2c[:, j:j + 1])
            nc.vector.tensor_tensor(out=s1[t][:, j, :], in0=h2t[:], in1=h2t[:],
                                    op=mult)
            nc.vector.tensor_scalar(out=s1[t][:, j, :], in0=s1[t][:, j, :],
                                    scalar1=-1.0, scalar2=1.0, op0=mult, op1=add)
            th = tmp3.tile([128, B], bf16, tag="th", name=f"th_{t}_{j}")
            nc.scalar.activation(th[:], pf[:], Tanh)
            prod = tmp3.tile([128, B], bf16, tag="prod", name=f"prod_{t}_{j}")
            nc.vector.tensor_tensor(out=prod[:], in0=pf[:],
                                    in1=s1[t][:, j, :], op=mult)
            nc.gpsimd.tensor_tensor(out=g1[t][:, j, :], in0=th[:], in1=prod[:],
                                    op=add)

        for jp in range(0, HC, 2):
            pbs = []
            for jj in range(jp, jp + 2):
                pb = mmp.tile([128, 2, B2], f32, tag="mm", name=f"pb_{t}_{jj}")
                for k in range(HC):
                    for i in range(2):
                        nc.tensor.matmul(
                            pb[:, i, :], w2t[:, k, jj * 128:(jj + 1) * 128],
                            h1[t][:, k, i * B2:(i + 1) * B2],
                            start=(k == 0 and i == 0),
                            stop=(k == 7 and i == 1),
                            skip_group_check=True)
                pbs.append(pb)
            pfs = []
            for jj in range(jp, jp + 2):
                pf = smallp.tile([128, 2, B2], f32, tag="sm",
                                 name=f"pf_{t}_{jj}")
                for i in range(2):
                    nc.tensor.matmul(pf[:, i, :],
                                     wf1a[:, jj * 128:(jj + 1) * 128],
                                     zu[t][:, i * B2:(i + 1) * B2],
                                     start=(i == 0), stop=(i == 1),
                                     skip_group_check=True)
                pfs.append(pf)
            b_post(jp, pbs[0], pfs[0])
            b_post(jp + 1, pbs[1], pfs[1])

    # s0 tiles allocated lazily with their tile's h1 (generation pairing)
    def alloc_s0(t):
        s0[t] = act2.tile([128, HC, B], bf16, tag="s0", name=f"s0_{t}")

    # tile 0: A standalone (+ its s0, computed immediately after h1)
    dma_in(0)
    if TILES > 1:
        dma_in(1)
    for j in range(HC):
        phase_A_single(0, j)
    alloc_s0(0)
    for j in range(HC):
        nc.vector.tensor_tensor(out=s0[0][:, j, :], in0=h1[0][:, j, :],
                                in1=h1[0][:, j, :], op=mult)
        nc.vector.tensor_scalar(out=s0[0][:, j, :], in0=s0[0][:, j, :],
                                scalar1=-1.0, scalar2=1.0, op0=mult, op1=add)
    phase_B(0)

    for it in range(1, TILES + 1):
        t_next = it if it < TILES else None
        if t_next is not None:
            alloc_s0(t_next)  # filled during phase_D below
        phase_E(it - 1, t_next)
        phase_D(it - 1)
        if t_next is not None:
            if t_next + 1 < TILES:
                dma_in(t_next + 1)
            phase_B(t_next)


def _prep_inputs(t, z, W1, b1, W2, b2, Wh, bh,
                 Wf1, bf1, Wf2, bf2, Wff, bff, Wp, bp):
    import ml_dtypes

    f = np.float32
    bf = ml_dtypes.bfloat16
    z = np.asarray(z, f)
    u = np.tanh(z @ np.asarray(Wp, f).T + np.asarray(bp, f))
    ones = np.ones((N, 1), f)
    zua = np.concatenate([z, ones, u], axis=1)   # [N, 21]

    W1 = np.asarray(W1, f)
    W2 = np.asarray(W2, f)
    Wh = np.asarray(Wh, f)
    Wf1 = np.asarray(Wf1, f)
    Wf2 = np.asarray(Wf2, f)
    Wff = np.asarray(Wff, f)

    def cb(x):
        return np.ascontiguousarray(np.asarray(x).astype(bf))

    def cf(x):
        return np.ascontiguousarray(np.asarray(x, f))

    w1a = np.concatenate([W1.T, np.asarray(b1, f)[None, :]], axis=0)   # [17,H]
    wf1t = Wf1.T                                                        # [20,H]
    wf1a = np.concatenate([wf1t[:D2], np.asarray(bf1, f)[None, :],
                           wf1t[D2:]], axis=0)                          # [21,H]
    w2nh = W2 * Wh[0][:, None]                                          # [j,k]
    w1nj = np.concatenate([W1[:, DQ:], -W1[:, :DQ]], axis=1)            # [H,16]
    wfft = Wff.T                                                        # [H,16]
    wffj = np.concatenate([wfft[:, DQ:], -wfft[:, :DQ]], axis=1)
    bffj = np.concatenate([np.asarray(bff, f)[DQ:], -np.asarray(bff, f)[:DQ]])

    shared = {
        "w1a": cb(w1a),
        "wf1a": cb(wf1a),
        "w2t": cb(W2.T),
        "w2nh": cb(w2nh),
        "wf2t": cb(Wf2.T),
        "w1nj": cb(w1nj),
        "wffj": cb(wffj),
        "b2c": cf(np.asarray(b2, f).reshape(HC, 128).T),
        "bf2c": cf(np.asarray(bf2, f).reshape(HC, 128).T),
        "bffj": cf(bffj.reshape(D2, 1)),
    }
    in_maps = []
    for r in range(NCORES):
        m = dict(shared)
        m["zua"] = cb(zua[r * NSH:(r + 1) * NSH].T)
        in_maps.append(m)
    return in_maps


def _postprocess(results):
    outs = [results[r]["st"].T for r in range(NCORES)]   # [NSH, 16] each
    return np.ascontiguousarray(np.concatenate(outs, axis=0).astype(np.float32))


def kernel(**inputs):
    global _BUILT
    from concourse.bass_utils import run_bass_kernel_spmd

    if _BUILT is None:
        _BUILT = _build()
    in_maps = _prep_inputs(**inputs)
    res = run_bass_kernel_spmd(_BUILT, in_maps, list(range(NCORES)))
    return _postprocess(res.results)
